# revision 34
# baseline (speedup 1.0000x reference)
"""MiniBindingAttention Trainium2 kernel.

Reference computation (per batch b, head h, T=2048, HD=64):
    Q = x_h * sign(bv_q); K = x_h * sign(bv_k); V = x_h * sign(bv_v)
    scores = Q @ K.T / sqrt(HD)
    attn   = causal ? sigmoid(4 * scores) : 0
    out    = attn @ V

Key algebra used here:
  - sigmoid(4*scale*QK) = sigmoid((x_q . x_k) * 0.5 * sq*sk) with sq*sk in {+-1}
    -> fold 0.5*sign(bv_q)*sign(bv_k) into one scaled copy of x (per-channel).
  - V's sign(bv_v) is applied per-channel to the OUTPUT (out^T layout keeps the
    channel dim on partitions, so it is a cheap per-partition scalar multiply).
  - scores are computed TRANSPOSED ([k, q] layout) so the second matmul
    (contraction over k) needs no on-chip transposes at all; the host supplies
    x both natural ([t, d], pre-swizzled per 128-row tile for contiguous DMA)
    and transposed+duplicated ([d, t] twice along partitions, so two k-tiles'
    score matmuls run concurrently in disjoint PE row-groups).
  - the second matmul is column-packed: even/odd k-tiles accumulate into
    partitions 0:64 / 64:128 of one PSUM bank; a DVE add folds the halves.
  - causal masking: DVE multiplies the post-sigmoid diagonal 128x128 block by
    a 0/1 staircase constant.

Sharding: B*H = 32 (batch, head) pairs, 4 per core across 8 cores.
"""

import numpy as np
import ml_dtypes

import concourse.bass as bass
import concourse.tile as tile
from concourse import bacc, mybir
from concourse.bass_utils import run_bass_kernel_spmd

N_CORES = 8
B, T, D, H, HD = 2, 2048, 1024, 16, 64
PAIRS = (B * H) // N_CORES  # 4 (b,h) pairs per core
KT = T // 128               # 16 k-tiles of 128 rows
QB = T // 512               # 4 q-blocks of 512 cols
F32 = mybir.dt.float32
F32R = mybir.dt.float32r
BF16 = mybir.dt.bfloat16
SIG = mybir.ActivationFunctionType.Sigmoid


def _round_fp32r(a: np.ndarray) -> np.ndarray:
    """Round fp32 to the hardware fp32r format (11-bit mantissa, RNE)."""
    v = a.astype(np.float32).view(np.uint32).astype(np.uint64)
    r = (v + 0x7FF + ((v >> 12) & 1)) & 0xFFFFF000
    return r.astype(np.uint32).view(np.float32)


def build():
    nc = bacc.Bacc("TRN2", target_bir_lowering=False)
    # xT duplicated along partitions: [0:64]=x^T, [64:128]=x^T (row-group pack)
    xT_d = nc.dram_tensor("xT", [PAIRS, 128, T], F32R, kind="ExternalInput")
    # wxT = xT * (0.5*sign(bv_q)*sign(bv_k)) precomputed on host
    wxT_d = nc.dram_tensor("wxT", [PAIRS, 128, T], F32R, kind="ExternalInput")
    # xN pre-swizzled on host: xN[p, pp, 64*k+d] = x[128*k+pp, d]
    xN_d = nc.dram_tensor("xN", [PAIRS, 128, KT * HD], BF16, kind="ExternalInput")
    # prm: [128, 2*PAIRS]; col p = 0.5*sign(bv_q)*sign(bv_k) (duplicated to
    # partitions 64:128 too); col PAIRS+p = sign(bv_v) (partitions 0:64 used)
    prm_d = nc.dram_tensor("prm", [128, 2 * PAIRS], F32, kind="ExternalInput")
    # stair01[p, n] = 0.0 if n < p else 1.0 (keep mask for diagonal blocks)
    msk_d = nc.dram_tensor("msk", [128, 128], BF16, kind="ExternalInput")
    out_d = nc.dram_tensor("outT", [PAIRS, HD, T], F32, kind="ExternalOutput")

    with tile.TileContext(nc) as tc:
        with (
            tc.tile_pool(name="consts", bufs=1) as consts,
            tc.tile_pool(name="xpool", bufs=2) as xpool,
            tc.tile_pool(name="attnp", bufs=4) as attnp,
            tc.tile_pool(name="outp", bufs=3) as outp,
            tc.tile_pool(name="psum_s", bufs=3, space="PSUM") as psum_s,
            tc.tile_pool(name="psum_o", bufs=1, space="PSUM") as psum_o,
        ):
            prm = consts.tile([128, 2 * PAIRS], F32)
            nc.sync.dma_start(out=prm, in_=prm_d[:])
            stair = consts.tile([128, 128], BF16)
            nc.sync.dma_start(out=stair, in_=msk_d[:])

            state = {}

            def load_pair(p):
                xT = xpool.tile([128, T], F32R, tag="xT")
                wxT = xpool.tile([128, T], F32R, tag="wxT")
                # chunked loads give the scheduler finer-grained dependencies;
                # pair 0's first chunk is further split across queues to cut
                # the cold-start latency of the very first matmul wave
                for c in range(4):
                    cs = slice(512 * c, 512 * c + 512)
                    if p == 0 and c == 0:
                        for base in (0, 32, 64, 96):
                            pr = slice(base, base + 32)
                            nc.sync.dma_start(
                                out=wxT[pr, cs], in_=wxT_d[p, pr, cs]
                            )
                            nc.sync.dma_start(out=xT[pr, cs], in_=xT_d[p, pr, cs])
                    else:
                        nc.sync.dma_start(out=wxT[:, cs], in_=wxT_d[p, :, cs])
                        nc.sync.dma_start(out=xT[:, cs], in_=xT_d[p, :, cs])
                xN = xpool.tile([128, KT * HD], BF16, tag="xN")
                nc.sync.dma_start(out=xN, in_=xN_d[p])
                state[p] = (xT, xN, wxT)

            waves = [
                (p, j, t, 2 * j + 2)
                for p in range(PAIRS)
                for j in range(QB)
                for t in range(2 * j + 2)
            ]

            oaccs = {}      # (p, j) -> [128, 512] psum accumulator
            pending = None  # deferred mm2 work: (p, j, t, nwave, att, i0)

            def emit_mm2(p, j, t, nwave, att, i0):
                _, xN, _ = state[p]
                oaccA, oaccB = oaccs[(p, j)]
                for sl, oacc in ((0, oaccA), (1, oaccB)):
                    i = i0 + sl
                    r = i - 4 * j
                    off = 128 * r if r > 0 else 0
                    nc.tensor.matmul(
                        out=oacc[64 * sl : 64 * sl + 64, off:512],
                        lhsT=xN[:, HD * i : HD * i + HD],
                        rhs=att[:, 512 * sl + off : 512 * (sl + 1)],
                        start=(t == 0),
                        stop=(t == nwave - 1),
                    )
                if t == nwave - 1:
                    # fold col-group halves with sign(bv_v) applied per channel:
                    # outs = (A * sv) + (B * sv); B is pre-scaled on its way to
                    # SBUF (tensor_tensor cannot read two PSUM operands)
                    sv = prm[0:HD, PAIRS + p : PAIRS + p + 1]
                    bs = outp.tile([HD, 512], F32, name="bs", tag="bs")
                    nc.vector.tensor_scalar_mul(bs, oaccB[64:128, :], sv)
                    outs = outp.tile([HD, 512], F32, name="outs", tag="outs")
                    nc.vector.scalar_tensor_tensor(
                        out=outs,
                        in0=oaccA[0:64, :],
                        scalar=sv,
                        in1=bs,
                        op0=mybir.AluOpType.mult,
                        op1=mybir.AluOpType.add,
                    )
                    # two half-height DMAs land on different queues
                    nc.sync.dma_start(
                        out=out_d[p, 0:32, 512 * j : 512 * j + 512], in_=outs[0:32]
                    )
                    nc.sync.dma_start(
                        out=out_d[p, 32:64, 512 * j : 512 * j + 512],
                        in_=outs[32:64],
                    )
                    del oaccs[(p, j)]

            for (p, j, t, nwave) in waves:
                if p not in state:
                    load_pair(p)
                if t == 0:
                    oaccA = psum_o.tile([128, 512], F32, name="oaccA", tag="oaccA")
                    oaccB = psum_o.tile([128, 512], F32, name="oaccB", tag="oaccB")
                    oaccs[(p, j)] = (oaccA, oaccB)
                    if j == 0:
                        # cols 0:128 of the odd col-group are never written
                        # (k-tile 1 is causally dead there); zero for the add
                        nc.vector.memset(oaccB[64:128, 0:128], 0.0)
                xT, xN, wxT = state[p]
                i0 = 2 * t
                sc = psum_s.tile([128, 1024], F32)
                att = attnp.tile([128, 1024], BF16)
                # --- scores^T for k-tiles i0, i0+1 (concurrent row-groups)
                for sl in (0, 1):
                    i = i0 + sl
                    bp = 64 * sl  # row-group base partition
                    nc.tensor.matmul(
                        out=sc[:, 512 * sl : 512 * sl + 512],
                        lhsT=wxT[bp : bp + 64, 128 * i : 128 * i + 128],
                        rhs=xT[bp : bp + 64, 512 * j : 512 * j + 512],
                        start=True,
                        stop=True,
                    )
                # --- sigmoid (trim fully-masked columns on diag tiles)
                if i0 + 1 < 4 * j:
                    nc.scalar.activation(out=att, in_=sc, func=SIG)
                else:
                    for sl in (0, 1):
                        r = i0 + sl - 4 * j
                        off = 128 * r if r > 0 else 0
                        nc.scalar.activation(
                            out=att[:, 512 * sl + off : 512 * (sl + 1)],
                            in_=sc[:, 512 * sl + off : 512 * (sl + 1)],
                            func=SIG,
                        )
                        # causal staircase on the diagonal 128x128 block
                        blk = slice(512 * sl + off, 512 * sl + off + 128)
                        nc.vector.tensor_tensor(
                            out=att[:, blk],
                            in0=att[:, blk],
                            in1=stair,
                            op=mybir.AluOpType.mult,
                        )
                # --- deferred second matmul from the previous wave
                if pending is not None:
                    emit_mm2(*pending)
                pending = (p, j, t, nwave, att, i0)
            emit_mm2(*pending)
    nc.compile()
    return nc


_CACHE: dict = {}


def _get_nc():
    if "nc" not in _CACHE:
        _CACHE["nc"] = build()
    return _CACHE["nc"]


def _make_in_maps(x, bv_q, bv_k, bv_v):
    x = np.asarray(x, np.float32)
    w = 0.5 * np.sign(bv_q).astype(np.float32) * np.sign(bv_k).astype(np.float32)
    sv = np.sign(bv_v).astype(np.float32)

    pi = np.arange(128)
    msk = (pi[None, :] >= pi[:, None]).astype(ml_dtypes.bfloat16)  # stair01[p, n]

    in_maps = []
    for c in range(N_CORES):
        xT = np.empty((PAIRS, 128, T), np.float32)
        wxT = np.empty((PAIRS, 128, T), np.float32)
        xN = np.empty((PAIRS, 128, KT * HD), ml_dtypes.bfloat16)
        prm = np.zeros((128, 2 * PAIRS), np.float32)
        for p in range(PAIRS):
            g = PAIRS * c + p
            b, h = divmod(g, H)
            xs = x[b, :, HD * h : HD * h + HD]  # [T, HD]
            # swizzle: xN[pp, 64*k+d] = xs[128*k+pp, d]
            xN[p] = xs.reshape(KT, 128, HD).transpose(1, 0, 2).reshape(128, KT * HD)
            xsT_r = _round_fp32r(xs.T)
            xT[p, 0:HD] = xsT_r
            xT[p, HD:128] = xsT_r
            # *(+-0.5) is exact: stays valid fp32r
            wxT[p, 0:HD] = xsT_r * w[h][:, None]
            wxT[p, HD:128] = wxT[p, 0:HD]
            prm[0:HD, p] = w[h]
            prm[HD:128, p] = w[h]
            prm[0:HD, PAIRS + p] = sv[h]
        in_maps.append(
            {
                "xT": xT,
                "wxT": wxT,
                "xN": xN,
                "prm": prm,
                "msk": msk,
            }
        )
    return in_maps


def _assemble(results):
    out = np.empty((B, T, D), np.float32)
    for c in range(N_CORES):
        oT = results[c]["outT"]  # [PAIRS, HD, T]
        for p in range(PAIRS):
            g = PAIRS * c + p
            b, h = divmod(g, H)
            out[b, :, HD * h : HD * h + HD] = oT[p].T
    return out


def _run(x, bv_q, bv_k, bv_v, **spmd_kwargs):
    in_maps = _make_in_maps(x, bv_q, bv_k, bv_v)
    res = run_bass_kernel_spmd(
        _get_nc(), in_maps, core_ids=list(range(N_CORES)), **spmd_kwargs
    )
    return _assemble(res.results), res


def kernel(x, bv_q, bv_k, bv_v):
    out, _ = _run(x, bv_q, bv_k, bv_v)
    return out


# revision 37
# speedup vs baseline: 1.2264x; 1.2264x over previous
"""MiniBindingAttention Trainium2 kernel.

Reference computation (per batch b, head h, T=2048, HD=64):
    Q = x_h * sign(bv_q); K = x_h * sign(bv_k); V = x_h * sign(bv_v)
    scores = Q @ K.T / sqrt(HD)
    attn   = causal ? sigmoid(4 * scores) : 0
    out    = attn @ V

Key algebra used here:
  - sigmoid(4*scale*QK) = sigmoid((x_q . x_k) * 0.5 * sq*sk) with sq*sk in {+-1}
    -> fold 0.5*sign(bv_q)*sign(bv_k) into one scaled copy of x (per-channel).
  - V's sign(bv_v) is applied per-channel to the OUTPUT (out^T layout keeps the
    channel dim on partitions, so it is a cheap per-partition scalar multiply).
  - scores are computed TRANSPOSED ([k, q] layout) so the second matmul
    (contraction over k) needs no on-chip transposes at all; the host supplies
    x both natural ([t, d], pre-swizzled per 128-row tile for contiguous DMA)
    and transposed+duplicated ([d, t] twice along partitions, so two k-tiles'
    score matmuls run concurrently in disjoint PE row-groups).
  - the second matmul is column-packed: even/odd k-tiles accumulate into
    partitions 0:64 / 64:128 of one PSUM bank; a DVE add folds the halves.
  - causal masking: DVE multiplies the post-sigmoid diagonal 128x128 block by
    a 0/1 staircase constant.

Sharding: B*H = 32 (batch, head) pairs, 4 per core across 8 cores.
"""

import numpy as np
import ml_dtypes

import concourse.bass as bass
import concourse.tile as tile
from concourse import bacc, mybir
from concourse.bass_utils import run_bass_kernel_spmd

N_CORES = 8
B, T, D, H, HD = 2, 2048, 1024, 16, 64
PAIRS = (B * H) // N_CORES  # 4 (b,h) pairs per core
KT = T // 128               # 16 k-tiles of 128 rows
QB = T // 512               # 4 q-blocks of 512 cols
F32 = mybir.dt.float32
F32R = mybir.dt.float32r
BF16 = mybir.dt.bfloat16
SIG = mybir.ActivationFunctionType.Sigmoid


def _round_fp32r(a: np.ndarray) -> np.ndarray:
    """Round fp32 to the hardware fp32r format (11-bit mantissa, RNE)."""
    v = a.astype(np.float32).view(np.uint32).astype(np.uint64)
    r = (v + 0x7FF + ((v >> 12) & 1)) & 0xFFFFF000
    return r.astype(np.uint32).view(np.float32)


def build():
    nc = bacc.Bacc("TRN2", target_bir_lowering=False)
    # xT duplicated along partitions: [0:64]=x^T, [64:128]=x^T (row-group pack)
    xT_d = nc.dram_tensor("xT", [PAIRS, 128, T], F32R, kind="ExternalInput")
    # wxT = xT * (0.5*sign(bv_q)*sign(bv_k)) precomputed on host
    wxT_d = nc.dram_tensor("wxT", [PAIRS, 128, T], F32R, kind="ExternalInput")
    # xN pre-swizzled on host: xN[p, pp, 64*k+d] = x[128*k+pp, d]
    xN_d = nc.dram_tensor("xN", [PAIRS, 128, KT * HD], BF16, kind="ExternalInput")
    # prm: [128, 2*PAIRS]; col p = 0.5*sign(bv_q)*sign(bv_k) (duplicated to
    # partitions 64:128 too); col PAIRS+p = sign(bv_v) (partitions 0:64 used)
    prm_d = nc.dram_tensor("prm", [128, 2 * PAIRS], F32, kind="ExternalInput")
    # stair01[p, n] = 0.0 if n < p else 1.0 (keep mask for diagonal blocks)
    msk_d = nc.dram_tensor("msk", [128, 128], BF16, kind="ExternalInput")
    out_d = nc.dram_tensor("outT", [PAIRS, HD, T], F32, kind="ExternalOutput")

    with tile.TileContext(nc) as tc:
        with (
            tc.tile_pool(name="consts", bufs=1) as consts,
            tc.tile_pool(name="xpool", bufs=2) as xpool,
            tc.tile_pool(name="attnp", bufs=4) as attnp,
            tc.tile_pool(name="outp", bufs=3) as outp,
            tc.tile_pool(name="psum_s", bufs=3, space="PSUM") as psum_s,
            tc.tile_pool(name="psum_o", bufs=1, space="PSUM") as psum_o,
        ):
            prm = consts.tile([128, 2 * PAIRS], F32)
            nc.sync.dma_start(out=prm, in_=prm_d[:])
            stair = consts.tile([128, 128], BF16)
            nc.sync.dma_start(out=stair, in_=msk_d[:])

            state = {}

            def load_pair(p):
                xT = xpool.tile([128, T], F32R, tag="xT")
                wxT = xpool.tile([128, T], F32R, tag="wxT")
                # chunked loads give the scheduler finer-grained dependencies
                for c in range(4):
                    cs = slice(512 * c, 512 * c + 512)
                    nc.sync.dma_start(out=wxT[:, cs], in_=wxT_d[p, :, cs])
                    nc.sync.dma_start(out=xT[:, cs], in_=xT_d[p, :, cs])
                xN = xpool.tile([128, KT * HD], BF16, tag="xN")
                nc.sync.dma_start(out=xN, in_=xN_d[p])
                state[p] = (xT, xN, wxT)

            waves = [
                (p, j, t, 2 * j + 2)
                for p in range(PAIRS)
                for j in range(QB)
                for t in range(2 * j + 2)
            ]

            oaccs = {}      # (p, j) -> [128, 512] psum accumulator
            pending = None  # deferred mm2 work: (p, j, t, nwave, att, i0)

            def emit_mm2(p, j, t, nwave, att, i0):
                _, xN, _ = state[p]
                oaccA, oaccB = oaccs[(p, j)]
                for sl, oacc in ((0, oaccA), (1, oaccB)):
                    i = i0 + sl
                    r = i - 4 * j
                    off = 128 * r if r > 0 else 0
                    nc.tensor.matmul(
                        out=oacc[64 * sl : 64 * sl + 64, off:512],
                        lhsT=xN[:, HD * i : HD * i + HD],
                        rhs=att[:, 512 * sl + off : 512 * (sl + 1)],
                        start=(t == 0),
                        stop=(t == nwave - 1),
                    )
                if t == nwave - 1:
                    # fold col-group halves with sign(bv_v) applied per channel:
                    # outs = (A * sv) + (B * sv); B is pre-scaled on its way to
                    # SBUF (tensor_tensor cannot read two PSUM operands)
                    sv = prm[0:HD, PAIRS + p : PAIRS + p + 1]
                    bs = outp.tile([HD, 512], F32, name="bs", tag="bs")
                    nc.vector.tensor_scalar_mul(bs, oaccB[64:128, :], sv)
                    outs = outp.tile([HD, 512], F32, name="outs", tag="outs")
                    nc.vector.scalar_tensor_tensor(
                        out=outs,
                        in0=oaccA[0:64, :],
                        scalar=sv,
                        in1=bs,
                        op0=mybir.AluOpType.mult,
                        op1=mybir.AluOpType.add,
                    )
                    nc.sync.dma_start(
                        out=out_d[p, :, 512 * j : 512 * j + 512], in_=outs
                    )
                    del oaccs[(p, j)]

            for (p, j, t, nwave) in waves:
                if p not in state:
                    load_pair(p)
                if t == 0:
                    oaccA = psum_o.tile([128, 512], F32, name="oaccA", tag="oaccA")
                    oaccB = psum_o.tile([128, 512], F32, name="oaccB", tag="oaccB")
                    oaccs[(p, j)] = (oaccA, oaccB)
                    if j == 0:
                        # cols 0:128 of the odd col-group are never written
                        # (k-tile 1 is causally dead there); zero for the add
                        nc.vector.memset(oaccB[64:128, 0:128], 0.0)
                xT, xN, wxT = state[p]
                i0 = 2 * t
                sc = psum_s.tile([128, 1024], F32)
                att = attnp.tile([128, 1024], BF16)
                # --- scores^T for k-tiles i0, i0+1 (concurrent row-groups)
                for sl in (0, 1):
                    i = i0 + sl
                    bp = 64 * sl  # row-group base partition
                    nc.tensor.matmul(
                        out=sc[:, 512 * sl : 512 * sl + 512],
                        lhsT=wxT[bp : bp + 64, 128 * i : 128 * i + 128],
                        rhs=xT[bp : bp + 64, 512 * j : 512 * j + 512],
                        start=True,
                        stop=True,
                    )
                # --- sigmoid (trim fully-masked columns on diag tiles)
                if i0 + 1 < 4 * j:
                    nc.scalar.activation(out=att, in_=sc, func=SIG)
                else:
                    for sl in (0, 1):
                        r = i0 + sl - 4 * j
                        off = 128 * r if r > 0 else 0
                        nc.scalar.activation(
                            out=att[:, 512 * sl + off : 512 * (sl + 1)],
                            in_=sc[:, 512 * sl + off : 512 * (sl + 1)],
                            func=SIG,
                        )
                        # causal staircase on the diagonal 128x128 block
                        blk = slice(512 * sl + off, 512 * sl + off + 128)
                        nc.vector.tensor_tensor(
                            out=att[:, blk],
                            in0=att[:, blk],
                            in1=stair,
                            op=mybir.AluOpType.mult,
                        )
                # --- deferred second matmul from the previous wave
                if pending is not None:
                    emit_mm2(*pending)
                pending = (p, j, t, nwave, att, i0)
            emit_mm2(*pending)
    nc.compile()
    return nc


_CACHE: dict = {}


def _get_nc():
    if "nc" not in _CACHE:
        _CACHE["nc"] = build()
    return _CACHE["nc"]


def _make_in_maps(x, bv_q, bv_k, bv_v):
    x = np.asarray(x, np.float32)
    w = 0.5 * np.sign(bv_q).astype(np.float32) * np.sign(bv_k).astype(np.float32)
    sv = np.sign(bv_v).astype(np.float32)

    pi = np.arange(128)
    msk = (pi[None, :] >= pi[:, None]).astype(ml_dtypes.bfloat16)  # stair01[p, n]

    in_maps = []
    for c in range(N_CORES):
        xT = np.empty((PAIRS, 128, T), np.float32)
        wxT = np.empty((PAIRS, 128, T), np.float32)
        xN = np.empty((PAIRS, 128, KT * HD), ml_dtypes.bfloat16)
        prm = np.zeros((128, 2 * PAIRS), np.float32)
        for p in range(PAIRS):
            g = PAIRS * c + p
            b, h = divmod(g, H)
            xs = x[b, :, HD * h : HD * h + HD]  # [T, HD]
            # swizzle: xN[pp, 64*k+d] = xs[128*k+pp, d]
            xN[p] = xs.reshape(KT, 128, HD).transpose(1, 0, 2).reshape(128, KT * HD)
            xsT_r = _round_fp32r(xs.T)
            xT[p, 0:HD] = xsT_r
            xT[p, HD:128] = xsT_r
            # *(+-0.5) is exact: stays valid fp32r
            wxT[p, 0:HD] = xsT_r * w[h][:, None]
            wxT[p, HD:128] = wxT[p, 0:HD]
            prm[0:HD, p] = w[h]
            prm[HD:128, p] = w[h]
            prm[0:HD, PAIRS + p] = sv[h]
        in_maps.append(
            {
                "xT": xT,
                "wxT": wxT,
                "xN": xN,
                "prm": prm,
                "msk": msk,
            }
        )
    return in_maps


def _assemble(results):
    out = np.empty((B, T, D), np.float32)
    for c in range(N_CORES):
        oT = results[c]["outT"]  # [PAIRS, HD, T]
        for p in range(PAIRS):
            g = PAIRS * c + p
            b, h = divmod(g, H)
            out[b, :, HD * h : HD * h + HD] = oT[p].T
    return out


def _run(x, bv_q, bv_k, bv_v, **spmd_kwargs):
    in_maps = _make_in_maps(x, bv_q, bv_k, bv_v)
    res = run_bass_kernel_spmd(
        _get_nc(), in_maps, core_ids=list(range(N_CORES)), **spmd_kwargs
    )
    return _assemble(res.results), res


def kernel(x, bv_q, bv_k, bv_v):
    out, _ = _run(x, bv_q, bv_k, bv_v)
    return out
